# revision 5
# baseline (speedup 1.0000x reference)
"""CocycleAttention TRN2 kernel (8 NeuronCores, SPMD).

Math (per batch b):
    Qp = Q @ Wq.T + bq ; Kp = K @ Wk.T + bk ; Vp = V @ Wv.T + bv
    pK = softmax(Kp, -1) ; ne[j] = sum_d pK*logpK (= -entropy)
    scores[i,j] = Qp[i,:].pK[j,:] - ne[j]   (the lseQ row-constant of the
        reference's cross term cancels in the row softmax)
    attn = softmax(scores, -1) ; out = attn @ Vp

Sharding: core c handles batch b=c//2, query rows h=(c%2)*1024..+1024.
K/V projections are recomputed per core (no collectives).

All matmuls run in float32r (1 cyc/row at N=512 vs 4 for fp32; max rel err
~1e-4 over K=1024 measured on HW).
"""

import numpy as np

import concourse.bass as bass
from concourse import bacc
import concourse.tile as tile
from concourse import mybir
from concourse import bass_utils
from concourse.masks import make_identity

F32 = mybir.dt.float32
F32R = mybir.dt.float32r
AF = mybir.ActivationFunctionType
OP = mybir.AluOpType
AX = mybir.AxisListType

B, NQ, NK, D = 4, 2048, 2048, 1024
NCORES = 8
NQC = NQ * B // NCORES          # 1024 query rows per core
NIT = NQC // 128                # 8 i-tiles
NJT = NK // 128                 # 16 j-tiles
NDC = D // 128                  # 8 contraction/d chunks


def build_nc():
    nc = bacc.Bacc("TRN2", target_bir_lowering=False, debug=False,
                   num_devices=NCORES)

    qt = nc.dram_tensor("qt", [D, NQC], F32R, kind="ExternalInput").ap()
    kt = nc.dram_tensor("kt", [D, NK], F32R, kind="ExternalInput").ap()
    vt = nc.dram_tensor("vt", [D, NK], F32R, kind="ExternalInput").ap()
    wqt = nc.dram_tensor("wqt", [D, D], F32R, kind="ExternalInput").ap()
    wkt = nc.dram_tensor("wkt", [D, D], F32R, kind="ExternalInput").ap()
    wvt = nc.dram_tensor("wvt", [D, D], F32R, kind="ExternalInput").ap()
    bq = nc.dram_tensor("bq", [NDC, 128, 1], F32, kind="ExternalInput").ap()
    bk = nc.dram_tensor("bk", [D], F32, kind="ExternalInput").ap()
    bv = nc.dram_tensor("bv", [D], F32, kind="ExternalInput").ap()

    attn_out = nc.dram_tensor("attn_out", [NQC, NK], F32, kind="ExternalOutput").ap()
    out_out = nc.dram_tensor("out_out", [NQC, D], F32, kind="ExternalOutput").ap()

    with tile.TileContext(nc) as tc:
        with (
            tc.tile_pool(name="singles", bufs=1) as singles,
            tc.tile_pool(name="dram", bufs=1, space="DRAM") as dram,
        ):
            ident = singles.tile([128, 128], F32)
            make_identity(nc, ident)

            bq_sb = singles.tile([128, NDC, 1], F32)
            nc.sync.dma_start(out=bq_sb, in_=bq.rearrange("c p x -> p c x"))
            bk_bcast = singles.tile([128, D], F32)
            nc.sync.dma_start(
                out=bk_bcast,
                in_=bass.AP(tensor=bk.tensor, offset=bk.offset, ap=[[0, 128], [1, D]]),
            )
            bv_bcast = singles.tile([128, D], F32)
            nc.sync.dma_start(
                out=bv_bcast,
                in_=bass.AP(tensor=bv.tensor, offset=bv.offset, ap=[[0, 128], [1, D]]),
            )

            ne_dram = dram.tile([NJT, 128, 1], F32)
            vp_spill = dram.tile([NK, D], F32R)
            at_spill = dram.tile([NJT, 128, NQC], F32R)

            with (
                tc.tile_pool(name="qpt", bufs=1) as qpt_pool,
                tc.tile_pool(name="pkt", bufs=1) as pkt_pool,
            ):
                qpt_sb = qpt_pool.tile([128, NDC, NQC], F32R)   # 32KB/part
                pkt_sb = pkt_pool.tile([128, NDC, NK], F32R)    # 64KB/part

                # ------------ Phase B: QpT = (Wq @ Q.T) + bq ------------
                with (
                    tc.tile_pool(name="b_w", bufs=1) as b_w,
                    tc.tile_pool(name="b_x", bufs=1) as b_x,
                    tc.tile_pool(name="b_ps", bufs=2, space="PSUM") as b_ps,
                ):
                    wq_sb = b_w.tile([128, NDC, D], F32R)
                    nc.sync.dma_start(out=wq_sb,
                                      in_=wqt.rearrange("(c p) d -> p c d", p=128))
                    qt_sb = b_x.tile([128, NDC, NQC], F32R)
                    nc.sync.dma_start(out=qt_sb,
                                      in_=qt.rearrange("(c p) i -> p c i", p=128))

                    for dc in range(NDC):
                        ps = b_ps.tile([128, NQC], F32)
                        for isl in range(NQC // 512):
                            for ec in range(NDC):
                                nc.tensor.matmul(
                                    ps[:, isl * 512:(isl + 1) * 512],
                                    wq_sb[:, ec, dc * 128:(dc + 1) * 128],
                                    qt_sb[:, ec, isl * 512:(isl + 1) * 512],
                                    start=(ec == 0), stop=(ec == NDC - 1),
                                )
                        nc.vector.tensor_scalar_add(
                            qpt_sb[:, dc, :], ps, bq_sb[:, dc, :])

                # ------------ Phase C1: Kp -> pK -> pKT, ne ------------
                with (
                    tc.tile_pool(name="c_w", bufs=1) as c_w,
                    tc.tile_pool(name="c_x", bufs=2) as c_x,
                    tc.tile_pool(name="c_ps", bufs=2, space="PSUM") as c_ps,
                    tc.tile_pool(name="c_t", bufs=2) as c_t,
                    tc.tile_pool(name="c_scr", bufs=3) as c_scr,
                    tc.tile_pool(name="c_pk", bufs=5) as c_pk,
                    tc.tile_pool(name="c_tp", bufs=2, space="PSUM") as c_tp,
                    tc.tile_pool(name="c_sm", bufs=8) as c_sm,
                ):
                    wk_sb = c_w.tile([128, NDC, D], F32R)
                    nc.sync.dma_start(out=wk_sb,
                                      in_=wkt.rearrange("(c p) d -> p c d", p=128))

                    pk_group = []
                    for jt in range(NJT):
                        kt_tile = c_x.tile([128, NDC, 128], F32R, tag="kt")
                        nc.sync.dma_start(
                            out=kt_tile,
                            in_=kt[:, jt * 128:(jt + 1) * 128].rearrange(
                                "(c p) j -> p c j", p=128),
                        )
                        ps = c_ps.tile([128, D], F32)
                        for ds in range(D // 512):
                            for ec in range(NDC):
                                nc.tensor.matmul(
                                    ps[:, ds * 512:(ds + 1) * 512],
                                    kt_tile[:, ec, :],
                                    wk_sb[:, ec, ds * 512:(ds + 1) * 512],
                                    start=(ec == 0), stop=(ec == NDC - 1),
                                )
                        # t = Kp + bk ; softmax over free dim
                        t_tile = c_t.tile([128, D], F32, tag="t")
                        nc.vector.tensor_add(t_tile, ps, bk_bcast)
                        nrm = c_sm.tile([128, 1], F32, tag="nrm")
                        nc.vector.tensor_reduce(out=nrm, in_=t_tile, axis=AX.X,
                                                op=OP.max, negate=True)
                        e_t = c_scr.tile([128, D], F32, tag="scr")
                        nc.scalar.activation(out=e_t, in_=t_tile, func=AF.Exp,
                                             bias=nrm, scale=1.0)
                        zsum = c_sm.tile([128, 1], F32, tag="z")
                        nc.vector.tensor_reduce(out=zsum, in_=e_t, axis=AX.X,
                                                op=OP.add)
                        rz = c_sm.tile([128, 1], F32, tag="rz")
                        nc.vector.reciprocal(rz, zsum)
                        pk_tile = c_pk.tile([128, D], F32, tag="pk")
                        nc.vector.tensor_scalar_mul(pk_tile, e_t, rz)
                        # ne = sum(pK*(t - max)) - lnZ ; sum(pK*t) via prod
                        prod = c_scr.tile([128, D], F32, tag="scr")
                        nc.vector.tensor_mul(prod, pk_tile, t_tile)
                        v2 = c_sm.tile([128, 1], F32, tag="v2")
                        nc.vector.tensor_reduce(out=v2, in_=prod, axis=AX.X,
                                                op=OP.add)
                        lz = c_sm.tile([128, 1], F32, tag="lz")
                        nc.scalar.activation(out=lz, in_=zsum, func=AF.Ln)
                        t1 = c_sm.tile([128, 1], F32, tag="t1")
                        nc.vector.tensor_add(t1, v2, nrm)      # sum(pK*t) - max
                        ne_sb = c_sm.tile([128, 1], F32, tag="ne")
                        nc.vector.tensor_sub(ne_sb, t1, lz)
                        nc.sync.dma_start(out=ne_dram[jt], in_=ne_sb)

                        # transpose pK in groups of 4 j-tiles -> pkt_sb
                        pk_group.append(pk_tile)
                        if len(pk_group) == 4:
                            j0 = (jt - 3) * 128
                            for dc in range(NDC):
                                pst = c_tp.tile([128, 512], F32)
                                for q in range(4):
                                    nc.tensor.transpose(
                                        pst[:, q * 128:(q + 1) * 128],
                                        pk_group[q][:, dc * 128:(dc + 1) * 128],
                                        ident,
                                    )
                                nc.scalar.copy(pkt_sb[:, dc, j0:j0 + 512], pst)
                            pk_group = []

                # ------------ Phase C2: Vp -> DRAM spill ------------
                with (
                    tc.tile_pool(name="v_w", bufs=1) as v_w,
                    tc.tile_pool(name="v_x", bufs=2) as v_x,
                    tc.tile_pool(name="v_ps", bufs=2, space="PSUM") as v_ps,
                    tc.tile_pool(name="v_o", bufs=3) as v_o,
                ):
                    wv_sb = v_w.tile([128, NDC, D], F32R)
                    nc.sync.dma_start(out=wv_sb,
                                      in_=wvt.rearrange("(c p) d -> p c d", p=128))
                    for jt in range(NJT):
                        vt_tile = v_x.tile([128, NDC, 128], F32R, tag="vt")
                        nc.sync.dma_start(
                            out=vt_tile,
                            in_=vt[:, jt * 128:(jt + 1) * 128].rearrange(
                                "(c p) j -> p c j", p=128),
                        )
                        ps = v_ps.tile([128, D], F32)
                        for ds in range(D // 512):
                            for ec in range(NDC):
                                nc.tensor.matmul(
                                    ps[:, ds * 512:(ds + 1) * 512],
                                    vt_tile[:, ec, :],
                                    wv_sb[:, ec, ds * 512:(ds + 1) * 512],
                                    start=(ec == 0), stop=(ec == NDC - 1),
                                )
                        vp_tile = v_o.tile([128, D], F32R)
                        nc.vector.tensor_add(vp_tile, ps, bv_bcast)
                        nc.sync.dma_start(
                            out=vp_spill[jt * 128:(jt + 1) * 128, :], in_=vp_tile)

                # ------------ Phase E1: S -> attn -> attnT spill ------------
                with (
                    tc.tile_pool(name="e_ne", bufs=1) as e_ne,
                    tc.tile_pool(name="e_ps", bufs=2, space="PSUM") as e_ps,
                    tc.tile_pool(name="e_sc", bufs=2) as e_sc,
                    tc.tile_pool(name="e_at", bufs=4) as e_at,
                    tc.tile_pool(name="e_tp", bufs=2, space="PSUM") as e_tp,
                    tc.tile_pool(name="e_st", bufs=3) as e_st,
                    tc.tile_pool(name="e_sm", bufs=8) as e_sm,
                ):
                    ne_bcast = e_ne.tile([128, NK], F32)
                    ne_flat = ne_dram[:]
                    nc.sync.dma_start(
                        out=ne_bcast,
                        in_=bass.AP(tensor=ne_flat.tensor, offset=ne_flat.offset,
                                    ap=[[0, 128], [1, NK]]),
                    )

                    at_group = []
                    for it in range(NIT):
                        scores = e_sc.tile([128, NK], F32, tag="scores")
                        for jh in range(2):
                            ps = e_ps.tile([128, 1024], F32)
                            for js in range(2):
                                for dc in range(NDC):
                                    nc.tensor.matmul(
                                        ps[:, js * 512:(js + 1) * 512],
                                        qpt_sb[:, dc, it * 128:(it + 1) * 128],
                                        pkt_sb[:, dc,
                                               jh * 1024 + js * 512:
                                               jh * 1024 + (js + 1) * 512],
                                        start=(dc == 0), stop=(dc == NDC - 1),
                                    )
                            nc.vector.tensor_sub(
                                scores[:, jh * 1024:(jh + 1) * 1024],
                                ps, ne_bcast[:, jh * 1024:(jh + 1) * 1024])
                        nsrm = e_sm.tile([128, 1], F32, tag="nsrm")
                        nc.vector.tensor_reduce(out=nsrm, in_=scores, axis=AX.X,
                                                op=OP.max, negate=True)
                        e_t2 = e_sc.tile([128, NK], F32, tag="et")
                        nc.scalar.activation(out=e_t2, in_=scores, func=AF.Exp,
                                             bias=nsrm, scale=1.0)
                        zs = e_sm.tile([128, 1], F32, tag="z")
                        nc.vector.tensor_reduce(out=zs, in_=e_t2, axis=AX.X,
                                                op=OP.add)
                        rz = e_sm.tile([128, 1], F32, tag="rz")
                        nc.vector.reciprocal(rz, zs)
                        attn_tile = e_at.tile([128, NK], F32, tag="attn")
                        nc.vector.tensor_scalar_mul(attn_tile, e_t2, rz)
                        nc.sync.dma_start(
                            out=attn_out[it * 128:(it + 1) * 128, :],
                            in_=attn_tile)

                        at_group.append(attn_tile)
                        if len(at_group) == 4:
                            i0 = (it - 3) * 128
                            for jc in range(NJT):
                                pst = e_tp.tile([128, 512], F32)
                                for q in range(4):
                                    nc.tensor.transpose(
                                        pst[:, q * 128:(q + 1) * 128],
                                        at_group[q][:, jc * 128:(jc + 1) * 128],
                                        ident,
                                    )
                                stg = e_st.tile([128, 512], F32R)
                                nc.scalar.copy(stg, pst)
                                nc.sync.dma_start(
                                    out=at_spill[jc, :, i0:i0 + 512], in_=stg)
                            at_group = []

            # ------------ Phase E2: out = attn @ Vp ------------
            with (
                tc.tile_pool(name="o_vp", bufs=1) as o_vp,
                tc.tile_pool(name="o_at", bufs=3) as o_at,
                tc.tile_pool(name="o_ps", bufs=2, space="PSUM") as o_ps,
                tc.tile_pool(name="o_o", bufs=3) as o_o,
            ):
                vp_sb = o_vp.tile([128, NJT, D], F32R)
                nc.sync.dma_start(
                    out=vp_sb,
                    in_=vp_spill[:].rearrange("(c p) d -> p c d", p=128))
                for it in range(NIT):
                    at_tile = o_at.tile([128, NJT, 128], F32R, tag="at")
                    nc.sync.dma_start(
                        out=at_tile,
                        in_=at_spill[:, :, it * 128:(it + 1) * 128].rearrange(
                            "c p i -> p c i"),
                    )
                    ps = o_ps.tile([128, D], F32)
                    for ds in range(D // 512):
                        for jc in range(NJT):
                            nc.tensor.matmul(
                                ps[:, ds * 512:(ds + 1) * 512],
                                at_tile[:, jc, :],
                                vp_sb[:, jc, ds * 512:(ds + 1) * 512],
                                start=(jc == 0), stop=(jc == NJT - 1),
                            )
                    out_tile = o_o.tile([128, D], F32)
                    nc.scalar.copy(out_tile, ps)
                    nc.sync.dma_start(
                        out=out_out[it * 128:(it + 1) * 128, :], in_=out_tile)

    nc.compile()
    return nc


_NC = None


def _get_nc():
    global _NC
    if _NC is None:
        _NC = build_nc()
    return _NC


def _prep_inputs(inputs):
    Q = np.asarray(inputs["Q"], np.float32)
    K = np.asarray(inputs["K"], np.float32)
    V = np.asarray(inputs["V"], np.float32)
    wqt = np.ascontiguousarray(np.asarray(inputs["Wq"], np.float32).T)
    wkt = np.ascontiguousarray(np.asarray(inputs["Wk"], np.float32).T)
    wvt = np.ascontiguousarray(np.asarray(inputs["Wv"], np.float32).T)
    bq = np.ascontiguousarray(
        np.asarray(inputs["bq"], np.float32).reshape(NDC, 128, 1))
    bk = np.asarray(inputs["bk"], np.float32)
    bv = np.asarray(inputs["bv"], np.float32)

    kts = [np.ascontiguousarray(K[b].T) for b in range(B)]
    vts = [np.ascontiguousarray(V[b].T) for b in range(B)]
    qts = [np.ascontiguousarray(Q[b].T) for b in range(B)]

    in_maps = []
    for c in range(NCORES):
        b, h = divmod(c, NCORES // B)
        in_maps.append({
            "qt": np.ascontiguousarray(qts[b][:, h * NQC:(h + 1) * NQC]),
            "kt": kts[b],
            "vt": vts[b],
            "wqt": wqt, "wkt": wkt, "wvt": wvt,
            "bq": bq, "bk": bk, "bv": bv,
        })
    return in_maps


def _run(inputs, trace=False):
    nc = _get_nc()
    in_maps = _prep_inputs(inputs)
    res = bass_utils.run_bass_kernel_spmd(
        nc, in_maps, core_ids=list(range(NCORES)), trace=trace)
    out = np.empty((B, NQ, D), np.float32)
    attn = np.empty((B, NQ, NK), np.float32)
    for c in range(NCORES):
        b, h = divmod(c, NCORES // B)
        r = res.results[c]
        out[b, h * NQC:(h + 1) * NQC, :] = r["out_out"]
        attn[b, h * NQC:(h + 1) * NQC, :] = r["attn_out"]
    return out, attn, res


def kernel(**inputs):
    out, attn, _ = _run(inputs)
    return out, attn
